# revision 9
# baseline (speedup 1.0000x reference)
"""MoE-LoRA Linear kernel for 8 Trainium2 NeuronCores (bf16 main + fp8 LoRA-down).

Sharding: core c -> (batch b = c//2, out-feature half = c%2); no
collectives — the router only needs this core's batch.

Structure: x (bf16, 16MB) is fully SBUF-resident. Chunk 0 arrives as
four 1MB d-quarter DMAs on the sync ring so the first main chain
starts as soon as ~1MB lands; chunks 1-3 arrive as d-halves split
across the sync and scalar rings so mid-stream bandwidth keeps the
PE fed. The LoRA down-projection h = A x (plus free-riding router
logit rows) runs as fp8-e4m3 DoubleRow chains at half cost, fed by
on-device bf16->fp8 vector casts into transient pair-packed tiles.
The softmax runs entirely on vector/scalar/sync (replicated DRAM
round-trip reads); gpsimd is a dedicated W-tile DMA channel. The
router-weighted LoRA + bias rows are folded into every o-tile's PSUM
accumulation (bf16), so there is no replay pass and no
read-modify-write output DMA.
"""
import sys

sys.path.insert(0, "/opt/trn_rl_repo")

import numpy as np
import ml_dtypes

import concourse.bass as bass
import concourse.mybir as mybir
import concourse.tile as tile
from concourse import bacc
from concourse.bass_utils import run_bass_kernel_spmd

F32 = mybir.dt.float32
BF16 = mybir.dt.bfloat16
F8 = mybir.dt.float8e4
BF16_NP = ml_dtypes.bfloat16
F8_NP = ml_dtypes.float8_e4m3

D, T, O_SH, E, R = 4096, 2048, 2048, 8, 8
ER = E * R          # 64 lora rows
ERE = ER + E        # 72 = lora rows + router logit rows
DT = D // 128       # 32 contraction chunks
DQ = DT // 4        # 8 per d-quarter (chunk-0 split)
DH = DT // 2        # 16 per d-half (chunks 1-3 split)
KP = DT // 2        # 16 DoubleRow k-pairs per h chain
EREP = 80           # ERE padded to a 16-byte fp8 stride for dual-fp8 LDW
TCH = T // 512      # 4 token chunks of 512
OT = O_SH // 128    # 16 out-feature tiles
ROUTER_TEMP = 1.0
SCALING = 16.0 / 8.0

_nc_cache = []


def build():
    nc = bacc.Bacc(None, target_bir_lowering=False)
    xh = nc.dram_tensor("xh", [TCH, 128, DT, 512], BF16, kind="ExternalInput")
    Wts = nc.dram_tensor("Wts", [OT, 128, DT, 128], BF16, kind="ExternalInput")
    atp8 = nc.dram_tensor("atp8", [128, KP, 2, EREP], F8, kind="ExternalInput")
    bta = nc.dram_tensor("bta", [ER + 1, O_SH], BF16, kind="ExternalInput")
    rb = nc.dram_tensor("rb", [E], F32, kind="ExternalInput")
    out = nc.dram_tensor("out", [O_SH, T], F32, kind="ExternalOutput")
    wscratch = nc.dram_tensor("wscratch", [E], F32)

    with tile.TileContext(nc) as tc:
        with (
            tc.tile_pool(name="big", bufs=1) as big,
            tc.tile_pool(name="wpool", bufs=3) as wpool,
            tc.tile_pool(name="x8p", bufs=3) as x8p,
            tc.tile_pool(name="ev", bufs=4) as evpool,
            tc.tile_pool(name="psm", bufs=6, space="PSUM") as psm,
            tc.tile_pool(name="psh", bufs=2, space="PSUM") as psh,
        ):
            # ---- x: chunk 0 as 4 d-quarters (sync); 1-3 as d-halves ----
            x0q = []
            for q in range(4):
                t_ = big.tile([128, DQ, 512], BF16, tag=f"x0q{q}")
                nc.sync.dma_start(t_[:], xh[0, :, q * DQ : (q + 1) * DQ, :])
                x0q.append(t_)
            # scalar ring head: atp8 (needed by first h chain)
            atp8_t = big.tile([128, KP, 2, EREP], F8, tag="atp8")
            nc.scalar.dma_start(atp8_t[:], atp8[:])
            xA, xB = [None], [None]
            for c in range(1, TCH):
                ta = big.tile([128, DH, 512], BF16, tag=f"xa{c}")
                nc.sync.dma_start(ta[:], xh[c, :, 0:DH, :])
                xA.append(ta)
                tb = big.tile([128, DH, 512], BF16, tag=f"xb{c}")
                nc.scalar.dma_start(tb[:], xh[c, :, DH:DT, :])
                xB.append(tb)
            bta_t = big.tile([ER + 1, O_SH], BF16, tag="bta")
            nc.sync.dma_start(bta_t[:], bta[:])
            rbt = big.tile([ERE, 1], F32, tag="rbt")
            nc.sync.dma_start(rbt[ER:ERE, :], rb[:, None])

            def xs(c, d):
                if c == 0:
                    return x0q[d // DQ][:, d % DQ, :]
                if d < DH:
                    return xA[c][:, d, :]
                return xB[c][:, d - DH, :]

            haug = big.tile([ER + 1, T], BF16, tag="haug")
            nc.vector.memset(haug[ER : ER + 1, :], 1.0)
            lg = big.tile([ERE, T], BF16, tag="lg")

            # ---- gpsimd queue: dedicated W-tile DMA channel ----
            def load_wt(o):
                wt = wpool.tile([128, DT, 128], BF16, tag="wt", name=f"wt{o}")
                nc.gpsimd.dma_start(wt[:], Wts[o])
                return wt

            wt0 = load_wt(0)

            # fp8 pair-packed x half-tiles for the DoubleRow h chains
            def cast_half(c, m):
                t_ = x8p.tile([128, KP // 2, 2, 512], F8, tag="x8", name=f"x8_{c}_{m}")
                for i in range(DH):
                    d = DH * m + i
                    nc.vector.tensor_copy(t_[:, i // 2, i % 2, :], xs(c, d))
                return t_

            x8h = {}
            x8h[(0, 0)] = cast_half(0, 0)
            x8h[(0, 1)] = cast_half(0, 1)

            # ---- o-tile 0 interleaved with fp8 h chains per x chunk ----
            ps0 = []
            for c in range(TCH):
                csl = slice(c * 512, (c + 1) * 512)
                p = psm.tile([128, 512], F32, tag="main", name=f"m0_{c}")
                for d in range(DT):
                    nc.tensor.matmul(
                        p[:], wt0[:, d, :], xs(c, d), start=(d == 0), stop=False
                    )
                ps0.append(p)
                if c + 1 < TCH:
                    x8h[(c + 1, 0)] = cast_half(c + 1, 0)
                    x8h[(c + 1, 1)] = cast_half(c + 1, 1)
                hps = psh.tile([EREP, 512], F32, tag="hps", name=f"h{c}")
                for k in range(KP):
                    nc.tensor.matmul(
                        hps[:],
                        atp8_t[:, k, :, :],
                        x8h[(c, k // (KP // 2))][:, k % (KP // 2), :, :],
                        start=(k == 0),
                        stop=(k == KP - 1),
                        perf_mode=mybir.MatmulPerfMode.DoubleRow,
                    )
                nc.vector.tensor_copy(haug[0:ER, csl], hps[0:ER, :])
                nc.vector.tensor_copy(lg[ER:ERE, csl], hps[ER:ERE, :])

            # ---- router: logits -> softmax -> scale haug rows ----
            lgr = big.tile([ERE, 1], F32, tag="lgr")
            nc.vector.reduce_sum(lgr[ER:ERE, :], lg[ER:ERE, :], axis=mybir.AxisListType.X)
            lg8 = big.tile([ERE, 1], F32, tag="lg8")
            nc.scalar.activation(
                lg8[ER:ERE, :], lgr[ER:ERE, :], mybir.ActivationFunctionType.Copy,
                scale=1.0 / (T * ROUTER_TEMP),
            )
            nc.vector.tensor_tensor(
                lg8[ER:ERE, :], lg8[ER:ERE, :], rbt[ER:ERE, :], mybir.AluOpType.add
            )
            # logits here are tiny (|l| < ~0.2): exp without max-subtraction.
            e8f = big.tile([ERE, 1], F32, tag="e8f")
            nc.scalar.activation(e8f[ER:ERE, :], lg8[ER:ERE, :], mybir.ActivationFunctionType.Exp)
            # replicate the 8 raw exps via a dram round trip: one read gives
            # each of 64 partitions its expert's exp, a second gives every
            # partition all 8 exps (for the softmax denominator).
            nc.sync.dma_start(wscratch[:], e8f[ER:ERE, 0])
            wexpf = big.tile([ER, 1], F32, tag="wexpf")
            wsrc = bass.AP(tensor=wscratch, offset=0, ap=[[1, E], [0, R]])
            nc.sync.dma_start(wexpf[:], wsrc)
            wall = big.tile([ER, E], F32, tag="wall")
            wsrc2 = bass.AP(tensor=wscratch, offset=0, ap=[[0, ER], [1, E]])
            nc.sync.dma_start(wall[:], wsrc2)
            ssb = big.tile([ER, 1], F32, tag="ssb")
            nc.vector.reduce_sum(ssb[:], wall[:], axis=mybir.AxisListType.X)
            rsb = big.tile([ER, 1], F32, tag="rsb")
            nc.vector.reciprocal(rsb[:], ssb[:])
            wexpn = big.tile([ER, 1], F32, tag="wexpn")
            nc.vector.tensor_tensor(wexpn[:], wexpf[:], rsb[:], mybir.AluOpType.mult)
            wexpb = big.tile([ER, 1], BF16, tag="wexpb")
            nc.vector.tensor_copy(wexpb[:], wexpn[:])
            nc.vector.tensor_tensor(
                haug[0:ER, :], haug[0:ER, :], wexpb[:].to_broadcast([ER, T]),
                mybir.AluOpType.mult,
            )

            def chain(o, wt, c, name):
                p = psm.tile([128, 512], F32, tag="main", name=name)
                for d in range(DT):
                    nc.tensor.matmul(
                        p[:], wt[:, d, :], xs(c, d), start=(d == 0), stop=False
                    )
                return p

            def aug(o, pstiles):
                osl = slice(o * 128, (o + 1) * 128)
                for c in range(TCH):
                    nc.tensor.matmul(
                        pstiles[c][:],
                        bta_t[:, osl],
                        haug[:, c * 512 : (c + 1) * 512],
                        start=False,
                        stop=True,
                    )

            def evict(o, pstiles):
                osl = slice(o * 128, (o + 1) * 128)
                for c in range(TCH):
                    ev = evpool.tile([128, 512], F32, tag="ev")
                    nc.vector.tensor_copy(ev[:], pstiles[c][:])
                    nc.scalar.dma_start(out[osl, c * 512 : (c + 1) * 512], ev[:])

            # ---- o-tile 1: first two chains before aug(o0) frees banks ----
            wt1 = load_wt(1)
            ps1 = [chain(1, wt1, c, f"m1_{c}") for c in (0, 1)]
            aug(0, ps0)
            evict(0, ps0)
            ps1 += [chain(1, wt1, c, f"m1_{c}") for c in (2, 3)]
            aug(1, ps1)
            evict(1, ps1)

            # ---- o-tiles 2..15 ----
            for o in range(2, OT):
                wt = load_wt(o)
                ps = [chain(o, wt, c, f"m{o}_{c}") for c in range(TCH)]
                aug(o, ps)
                evict(o, ps)

    nc.compile()
    return nc


def _get_nc():
    if not _nc_cache:
        _nc_cache.append(build())
    return _nc_cache[0]


def kernel(x, W_base, b_base, lora_A, lora_B, router_W, router_b):
    x = np.asarray(x, dtype=np.float32)
    W_base = np.asarray(W_base, dtype=np.float32)
    b_base = np.asarray(b_base, dtype=np.float32)
    lora_A = np.asarray(lora_A, dtype=np.float32)
    lora_B = np.asarray(lora_B, dtype=np.float32)
    router_W = np.asarray(router_W, dtype=np.float32)
    router_b = np.asarray(router_b, dtype=np.float32)

    B, S, D_ = x.shape
    O = W_base.shape[0]

    M = np.concatenate([lora_A.reshape(ER, D_), router_W], axis=0)  # [72, D]
    M80 = np.zeros((EREP, D_), dtype=np.float32)
    M80[:ERE] = M
    atp8_h = np.ascontiguousarray(
        M80.reshape(EREP, KP, 2, 128).transpose(3, 1, 2, 0).astype(F8_NP)
    )
    rb_h = router_b.astype(np.float32)

    xh_b = []
    for b in range(B):
        xh_b.append(
            np.ascontiguousarray(
                x[b].reshape(TCH, 512, DT, 128).transpose(0, 3, 2, 1).astype(BF16_NP)
            )
        )
    Wts_h, bta_h = [], []
    for half in range(2):
        osl = slice(half * O_SH, (half + 1) * O_SH)
        Wts_h.append(
            np.ascontiguousarray(
                W_base[osl].reshape(OT, 128, DT, 128).transpose(0, 3, 2, 1).astype(BF16_NP)
            )
        )
        bta_lo = SCALING * lora_B[:, osl, :].transpose(0, 2, 1).reshape(ER, O_SH)
        bta_h.append(
            np.ascontiguousarray(
                np.concatenate([bta_lo, b_base[osl][None, :]], axis=0).astype(BF16_NP)
            )
        )

    in_maps = []
    for c in range(8):
        b, half = c // 2, c % 2
        in_maps.append(
            {
                "xh": xh_b[b],
                "Wts": Wts_h[half],
                "atp8": atp8_h,
                "bta": bta_h[half],
                "rb": rb_h,
            }
        )

    global _last_in_maps
    _last_in_maps = in_maps
    nc = _get_nc()
    res = run_bass_kernel_spmd(nc, in_maps, core_ids=list(range(8)))
    out = np.empty((B, S, O), dtype=np.float32)
    for c in range(8):
        b, half = c // 2, c % 2
        out[b, :, half * O_SH : (half + 1) * O_SH] = res.results[c]["out"].T
    return out


# revision 10
# speedup vs baseline: 1.0196x; 1.0196x over previous
"""MoE-LoRA Linear kernel for 8 Trainium2 NeuronCores (bf16) — v2 (proven 522865 ns).

Sharding: core c -> (batch b = c//2, out-feature half = c%2); no
collectives — the router only needs this core's batch.
"""
import sys

sys.path.insert(0, "/opt/trn_rl_repo")

import numpy as np
import ml_dtypes

import concourse.bass as bass
import concourse.mybir as mybir
import concourse.tile as tile
from concourse import bacc
from concourse.bass_utils import run_bass_kernel_spmd

F32 = mybir.dt.float32
BF16 = mybir.dt.bfloat16
BF16_NP = ml_dtypes.bfloat16

D, T, O_SH, E, R = 4096, 2048, 2048, 8, 8
ER = E * R          # 64 lora rows
ERE = ER + E        # 72 = lora rows + router logit rows
DT = D // 128       # 32 contraction chunks
TCH = T // 512      # 4 token chunks of 512
OT = O_SH // 128    # 16 out-feature tiles
ROUTER_TEMP = 1.0
SCALING = 16.0 / 8.0

_nc_cache = []


def build():
    nc = bacc.Bacc(None, target_bir_lowering=False)
    xh = nc.dram_tensor("xh", [TCH, 128, DT, 512], BF16, kind="ExternalInput")
    Wts = nc.dram_tensor("Wts", [OT, 128, DT, 128], BF16, kind="ExternalInput")
    atp = nc.dram_tensor("atp", [128, DT, ERE], BF16, kind="ExternalInput")
    bta = nc.dram_tensor("bta", [ER + 1, O_SH], BF16, kind="ExternalInput")
    rb = nc.dram_tensor("rb", [E], F32, kind="ExternalInput")
    out = nc.dram_tensor("out", [O_SH, T], F32, kind="ExternalOutput")
    wscratch = nc.dram_tensor("wscratch", [E], F32)

    with tile.TileContext(nc) as tc:
        with (
            tc.tile_pool(name="big", bufs=1) as big,
            tc.tile_pool(name="wpool", bufs=3) as wpool,
            tc.tile_pool(name="ev", bufs=4) as evpool,
            tc.tile_pool(name="psm", bufs=6, space="PSUM") as psm,
            tc.tile_pool(name="psh", bufs=2, space="PSUM") as psh,
        ):
            # ---- load order on sync queue: atp, x chunks, bta, rb ----
            atp_t = big.tile([128, DT, ERE], BF16, tag="atp")
            nc.sync.dma_start(atp_t[:], atp[:])
            xsb = []
            for c in range(TCH):
                xt = big.tile([128, DT, 512], BF16, tag=f"x{c}")
                nc.sync.dma_start(xt[:], xh[c])
                xsb.append(xt)
            bta_t = big.tile([ER + 1, O_SH], BF16, tag="bta")
            nc.sync.dma_start(bta_t[:], bta[:])
            rbt = big.tile([ERE, 1], F32, tag="rbt")
            nc.sync.dma_start(rbt[ER:ERE, :], rb[:, None])

            haug = big.tile([ER + 1, T], BF16, tag="haug")
            nc.vector.memset(haug[ER : ER + 1, :], 1.0)
            ones_t = big.tile([ERE, ER], BF16, tag="ones")
            nc.vector.memset(ones_t[ER:ERE, :], 1.0)
            lg = big.tile([ERE, T], F32, tag="lg")

            # ---- gpsimd queue: dedicated W-tile DMA channel ----
            def load_wt(o):
                wt = wpool.tile([128, DT, 128], BF16, tag="wt", name=f"wt{o}")
                nc.gpsimd.dma_start(wt[:], Wts[o])
                return wt

            wt0 = load_wt(0)

            # ---- o-tile 0 interleaved with h chains per x chunk ----
            ps0 = []
            for c in range(TCH):
                csl = slice(c * 512, (c + 1) * 512)
                hps = psh.tile([ERE, 512], F32, tag="hps", name=f"h{c}")
                for d in range(DT):
                    nc.tensor.matmul(
                        hps[:],
                        atp_t[:, d, :],
                        xsb[c][:, d, :],
                        start=(d == 0),
                        stop=(d == DT - 1),
                    )
                nc.vector.tensor_copy(haug[0:ER, csl], hps[0:ER, :])
                nc.vector.tensor_copy(lg[ER:ERE, csl], hps[ER:ERE, :])
                p = psm.tile([128, 512], F32, tag="main", name=f"m0_{c}")
                for d in range(DT):
                    nc.tensor.matmul(
                        p[:],
                        wt0[:, d, :],
                        xsb[c][:, d, :],
                        start=(d == 0),
                        stop=False,
                    )
                ps0.append(p)

            # ---- router: logits -> softmax -> scale haug rows ----
            lgr = big.tile([ERE, 1], F32, tag="lgr")
            nc.vector.reduce_sum(lgr[ER:ERE, :], lg[ER:ERE, :], axis=mybir.AxisListType.X)
            lg8 = big.tile([ERE, 1], F32, tag="lg8")
            nc.scalar.activation(
                lg8[ER:ERE, :], lgr[ER:ERE, :], mybir.ActivationFunctionType.Copy,
                scale=1.0 / (T * ROUTER_TEMP),
            )
            nc.vector.tensor_tensor(
                lg8[ER:ERE, :], lg8[ER:ERE, :], rbt[ER:ERE, :], mybir.AluOpType.add
            )
            # logits here are tiny (|l| < ~0.2): exp without max-subtraction.
            e8f = big.tile([ERE, 1], F32, tag="e8f")
            nc.scalar.activation(e8f[ER:ERE, :], lg8[ER:ERE, :], mybir.ActivationFunctionType.Exp)
            e8b = big.tile([ERE, 1], BF16, tag="e8b")
            nc.vector.tensor_copy(e8b[ER:ERE, :], e8f[ER:ERE, :])
            # sum exp over the 8 experts AND broadcast to partitions 0-63
            # in one matmul: ones[8,64].T @ e8[8,1] -> [64,1].
            sps = psh.tile([ER, 1], F32, tag="hps", name="sps")
            nc.tensor.matmul(
                sps[:], ones_t[ER:ERE, :], e8b[ER:ERE, :], start=True, stop=True
            )
            ssb = big.tile([ER, 1], F32, tag="ssb")
            nc.vector.tensor_copy(ssb[:], sps[:])
            rsb = big.tile([ER, 1], F32, tag="rsb")
            nc.vector.reciprocal(rsb[:], ssb[:])
            # replicate the 8 raw exps to 64 rows via a dram round trip
            nc.sync.dma_start(wscratch[:], e8f[ER:ERE, 0])
            wexpf = big.tile([ER, 1], F32, tag="wexpf")
            wsrc = bass.AP(tensor=wscratch, offset=0, ap=[[1, E], [0, R]])
            nc.sync.dma_start(wexpf[:], wsrc)
            wexpn = big.tile([ER, 1], F32, tag="wexpn")
            nc.vector.tensor_tensor(wexpn[:], wexpf[:], rsb[:], mybir.AluOpType.mult)
            wexpb = big.tile([ER, 1], BF16, tag="wexpb")
            nc.vector.tensor_copy(wexpb[:], wexpn[:])
            nc.vector.tensor_tensor(
                haug[0:ER, :], haug[0:ER, :], wexpb[:].to_broadcast([ER, T]),
                mybir.AluOpType.mult,
            )

            def chain(o, wt, c, name):
                p = psm.tile([128, 512], F32, tag="main", name=name)
                for d in range(DT):
                    nc.tensor.matmul(
                        p[:],
                        wt[:, d, :],
                        xsb[c][:, d, :],
                        start=(d == 0),
                        stop=False,
                    )
                return p

            def aug(o, pstiles):
                osl = slice(o * 128, (o + 1) * 128)
                for c in range(TCH):
                    nc.tensor.matmul(
                        pstiles[c][:],
                        bta_t[:, osl],
                        haug[:, c * 512 : (c + 1) * 512],
                        start=False,
                        stop=True,
                    )

            def evict(o, pstiles):
                osl = slice(o * 128, (o + 1) * 128)
                for c in range(TCH):
                    ev = evpool.tile([128, 512], F32, tag="ev")
                    nc.vector.tensor_copy(ev[:], pstiles[c][:])
                    nc.scalar.dma_start(out[osl, c * 512 : (c + 1) * 512], ev[:])

            # ---- o-tile 1: first two chains before aug(o0) frees banks ----
            wt1 = load_wt(1)
            ps1 = [chain(1, wt1, c, f"m1_{c}") for c in (0, 1)]
            aug(0, ps0)
            evict(0, ps0)
            ps1 += [chain(1, wt1, c, f"m1_{c}") for c in (2, 3)]
            aug(1, ps1)
            evict(1, ps1)

            # ---- o-tiles 2..15 ----
            for o in range(2, OT):
                wt = load_wt(o)
                ps = [chain(o, wt, c, f"m{o}_{c}") for c in range(TCH)]
                aug(o, ps)
                evict(o, ps)

    nc.compile()
    return nc


def _get_nc():
    if not _nc_cache:
        _nc_cache.append(build())
    return _nc_cache[0]


def kernel(x, W_base, b_base, lora_A, lora_B, router_W, router_b):
    x = np.asarray(x, dtype=np.float32)
    W_base = np.asarray(W_base, dtype=np.float32)
    b_base = np.asarray(b_base, dtype=np.float32)
    lora_A = np.asarray(lora_A, dtype=np.float32)
    lora_B = np.asarray(lora_B, dtype=np.float32)
    router_W = np.asarray(router_W, dtype=np.float32)
    router_b = np.asarray(router_b, dtype=np.float32)

    B, S, D_ = x.shape
    O = W_base.shape[0]

    M = np.concatenate([lora_A.reshape(ER, D_), router_W], axis=0)  # [72, D]
    atp_h = np.ascontiguousarray(
        M.reshape(ERE, DT, 128).transpose(2, 1, 0).astype(BF16_NP)
    )
    rb_h = router_b.astype(np.float32)

    xh_b = []
    for b in range(B):
        xh_b.append(
            np.ascontiguousarray(
                x[b].reshape(TCH, 512, DT, 128).transpose(0, 3, 2, 1).astype(BF16_NP)
            )
        )
    Wts_h, bta_h = [], []
    for half in range(2):
        osl = slice(half * O_SH, (half + 1) * O_SH)
        Wts_h.append(
            np.ascontiguousarray(
                W_base[osl].reshape(OT, 128, DT, 128).transpose(0, 3, 2, 1).astype(BF16_NP)
            )
        )
        bta_lo = SCALING * lora_B[:, osl, :].transpose(0, 2, 1).reshape(ER, O_SH)
        bta_h.append(
            np.ascontiguousarray(
                np.concatenate([bta_lo, b_base[osl][None, :]], axis=0).astype(BF16_NP)
            )
        )

    in_maps = []
    for c in range(8):
        b, half = c // 2, c % 2
        in_maps.append(
            {
                "xh": xh_b[b],
                "Wts": Wts_h[half],
                "atp": atp_h,
                "bta": bta_h[half],
                "rb": rb_h,
            }
        )

    global _last_in_maps
    _last_in_maps = in_maps
    nc = _get_nc()
    res = run_bass_kernel_spmd(nc, in_maps, core_ids=list(range(8)))
    out = np.empty((B, S, O), dtype=np.float32)
    for c in range(8):
        b, half = c // 2, c % 2
        out[b, :, half * O_SH : (half + 1) * O_SH] = res.results[c]["out"].T
    return out
